# revision 1
# baseline (speedup 1.0000x reference)
"""Canny filter (nn_CannyFilter) Trainium2 Bass kernel.

Math: the reference pipeline collapses to
    s   = mean_c img                       (channel mean; done by DMA accumulate)
    b   = gauss3x3 (*) s                   (separable: [g0,g1,g0] x [g0,g1,g0])
    gx  = sobelx (*) b = [.5,1,.5]_col (x) [-1,0,1]_row
    gy  = sobely (*) b = [-1,0,1]_col (x) [.5,1,.5]_row
    gm  = sqrt(gx^2 + gy^2)
    t1  = ring (*) gm                      (ring = 3x3 ones minus center)
    out = ring (*) t1
(each conv zero-padded by 1; the 8 directional one-hot kernels sum to `ring`,
and the NMS conv over the 8 directions is the same `ring` again).

Layout: each 512x512 plane is ONE SBUF tile [128, 2048]: partition p holds
image rows 4p..4p+3 (each row = 512 contiguous floats). W-direction convs are
free-dim shifts (DVE); H-direction convs are shifted-diagonal matmuls on the
PE (fp32r), with no partition-halo problems since the whole plane is in-tile.

Sharding: pure data parallel, 4 images per core across 8 cores.
"""
import numpy as np
from contextlib import ExitStack

import concourse.bass as bass
import concourse.tile as tile
from concourse import bacc, mybir
from concourse.bass_utils import run_bass_kernel_spmd

N_CORES = 8
B_TOTAL = 32
B_PER = B_TOTAL // N_CORES  # 4 images per core
C, H, W = 3, 512, 512
P = 128          # SBUF partitions
RPP = H // P     # 4 rows per partition
FW = RPP * W     # 2048 free dim per plane

F32 = mybir.dt.float32
F32R = mybir.dt.float32r
AF = mybir.ActivationFunctionType
OP = mybir.AluOpType

# gaussian separable vector (mu=0, sigma=1, k=3 -> exactly separable)
_g1 = np.linspace(-1.0, 1.0, 3)
_gg = np.exp(-(_g1[None, :] ** 2 + _g1[:, None] ** 2) / 2.0) / (2.0 * np.pi)
_gg = _gg / _gg.sum()
_u, _s, _vt = np.linalg.svd(_gg)
_gv = np.abs(_u[:, 0]) * np.sqrt(_s[0])
G0, G1 = float(_gv[0]), float(_gv[1])


def _plane_view(dram_ap):
    """[H, W] dram AP -> [128, 2048] (partition p = rows 4p..4p+3)."""
    return dram_ap.rearrange("(p q) w -> p (q w)", q=RPP)


def _build_nc():
    nc = bacc.Bacc("TRN2", target_bir_lowering=False, debug=False,
                   num_devices=N_CORES)
    img_d = nc.dram_tensor("img", [B_PER, C, H, W], F32, kind="ExternalInput")
    out_d = nc.dram_tensor("out", [B_PER, H, W], F32, kind="ExternalOutput")

    with tile.TileContext(nc, pool_alloc_mode="queue") as tc, ExitStack() as ctx:
        cpool = ctx.enter_context(tc.tile_pool(name="consts", bufs=1))
        spool = ctx.enter_context(tc.tile_pool(name="splanes", bufs=3))
        bpool = ctx.enter_context(tc.tile_pool(name="bpl", bufs=3))
        ppool = ctx.enter_context(tc.tile_pool(name="planes", bufs=1))
        ppool2 = ctx.enter_context(tc.tile_pool(name="planes2", bufs=2))
        upool = ctx.enter_context(tc.tile_pool(name="utmp", bufs=1))
        opool = ctx.enter_context(tc.tile_pool(name="outs", bufs=2))
        psum = ctx.enter_context(tc.tile_pool(name="ps", bufs=4, space="PSUM"))

        # ---- shifted-diagonal lhsT constants ------------------------------
        # D[p, f] = f - p  (int32), then lhsT = (D == delta) * coef  (fp32r)
        dmat = cpool.tile([P, P], mybir.dt.int32)
        nc.gpsimd.iota(dmat[:], pattern=[[1, P]], base=0, channel_multiplier=-1)

        lhs_cache = {}

        def lhs(delta, coef):
            key = (delta, round(coef, 10))
            if key not in lhs_cache:
                t = cpool.tile([P, P], F32R, tag=f"lhs_{len(lhs_cache)}")
                nc.vector.tensor_scalar(t[:], dmat[:], float(delta), float(coef),
                                        OP.is_equal, OP.mult)
                lhs_cache[key] = t
            return lhs_cache[key]

        # H-direction conv as matmuls.  out row r=4p+c taps row r+dr:
        #   cc = c+dr in 0..3  -> same partition, block cc, diag delta=0
        #   cc = -1            -> partition p-1, block 3, lhsT delta=+1
        #   cc = 4             -> partition p+1, block 0, lhsT delta=-1
        def col_mm(ps_t, inputs, c0=0, c1=RPP):
            """inputs: list of (tile, taps); accumulate all H-conv taps into ps_t.
            Writes blocks c0..c1-1 into ps_t at local offsets."""
            for c in range(c0, c1):
                todo = []
                for x_t, taps in inputs:
                    xv = x_t[:]
                    for dr, coef in taps:
                        cc = c + dr
                        if cc == -1:
                            todo.append((lhs(+1, coef), xv, 3))
                        elif cc == RPP:
                            todo.append((lhs(-1, coef), xv, 0))
                        else:
                            todo.append((lhs(0, coef), xv, cc))
                for j, (lt, xv, sb) in enumerate(todo):
                    lc = c - c0
                    nc.tensor.matmul(
                        ps_t[:, lc * W:(lc + 1) * W], lt[:],
                        xv[:, sb * W:(sb + 1) * W],
                        start=(j == 0), stop=(j == len(todo) - 1))


        def v3(t):
            return t[:].rearrange("p (c w) -> p c w", w=W)

        # W-direction symmetric neighbor sum u[w] = x[w-1] + x[w+1]
        def row_u(x_t, engine, tag):
            u = upool.tile([P, FW], F32, tag=tag)
            uv, xv = v3(u), v3(x_t)
            engine.tensor_add(uv[:, :, 1:W - 1], xv[:, :, 0:W - 2], xv[:, :, 2:W])
            engine.tensor_copy(uv[:, :, 0:1], xv[:, :, 1:2])
            engine.tensor_copy(uv[:, :, W - 1:W], xv[:, :, W - 2:W - 1])
            return u

        s_tiles = []
        for _si in range(B_PER):
            s_i = spool.tile([P, FW], F32, tag="s")
            s_tiles.append(s_i)
        for ch in range(C):
            for i in range(B_PER):
                nc.gpsimd.dma_start(
                    s_tiles[i][:], _plane_view(img_d[i, ch]),
                    accum_op=(OP.bypass if ch == 0 else OP.add))

        from contextlib import nullcontext
        for i in range(B_PER):
          with (tc.high_priority(offset=i * 180) if i else nullcontext()):
            s = s_tiles[i]

            # ---- gauss: r1 = Grow(s)/g0 ; b = Gcol(g0*r1)/3 ---------------
            u = row_u(s, nc.vector, "ug")
            r1 = ppool.tile([P, FW], F32R, tag="r1")
            nc.vector.scalar_tensor_tensor(r1[:], s[:], G1 / G0, u[:],
                                           OP.mult, OP.add)
            m = G0 * G0 / 3.0
            gtaps = [(r1, [(-1, m), (0, G1 * G0 / 3.0), (1, m)])]
            b = bpool.tile([P, FW], F32R, tag="b")
            for h in range(2):
                ps_b = psum.tile([P, FW // 2], F32, tag="ps")
                col_mm(ps_b, gtaps, c0=2 * h, c1=2 * h + 2)
                nc.scalar.activation(b[:, h * (FW // 2):(h + 1) * (FW // 2)],
                                     ps_b[:], AF.Copy)

            # ---- sobel x: d = b[w+1]-b[w-1] ; gx = Acol(d) ----------------
            d = ppool.tile([P, FW], F32R, tag="d")
            dv, bv = v3(d), v3(b)
            nc.vector.tensor_sub(dv[:, :, 1:W - 1], bv[:, :, 2:W], bv[:, :, 0:W - 2])
            nc.vector.tensor_copy(dv[:, :, 0:1], bv[:, :, 1:2])
            nc.vector.tensor_scalar_mul(dv[:, :, W - 1:W], bv[:, :, W - 2:W - 1], -1.0)
            q1 = ppool.tile([P, FW], F32, tag="q1")
            for h in range(2):
                ps_gx = psum.tile([P, FW // 2], F32, tag="ps")
                col_mm(ps_gx, [(d, [(-1, 0.5), (0, 1.0), (1, 0.5)])], 2 * h, 2 * h + 2)
                nc.scalar.activation(q1[:, h * (FW // 2):(h + 1) * (FW // 2)],
                                     ps_gx[:], AF.Square)

            # ---- sobel y: a = 2b + u2 (=Arow(b)/0.5); gy = 0.5*Dcol(a) ----
            u2 = row_u(b, nc.gpsimd, "ua")
            a = ppool.tile([P, FW], F32R, tag="a")
            nc.vector.scalar_tensor_tensor(a[:], b[:], 2.0, u2[:], OP.mult, OP.add)
            q2 = ppool.tile([P, FW], F32, tag="q2")
            for h in range(2):
                ps_gy = psum.tile([P, FW // 2], F32, tag="ps")
                col_mm(ps_gy, [(a, [(-1, -0.5), (1, 0.5)])], 2 * h, 2 * h + 2)
                nc.scalar.activation(q2[:, h * (FW // 2):(h + 1) * (FW // 2)],
                                     ps_gy[:], AF.Square)

            # ---- gm = sqrt(q1 + q2) --------------------------------------
            nc.vector.tensor_add(q1[:], q1[:], q2[:])
            gm = ppool2.tile([P, FW], F32R, tag="gm")
            nc.scalar.activation(gm[:], q1[:], AF.Sqrt)

            # ---- ring 1: t1 = Bcol(Brow(gm)) - gm ------------------------
            u3 = row_u(gm, nc.gpsimd, "ub1")
            br = ppool.tile([P, FW], F32R, tag="br")
            nc.vector.tensor_add(br[:], u3[:], gm[:])
            t1 = ppool2.tile([P, FW], F32R, tag="t1")
            for h in range(2):
                ps_t1 = psum.tile([P, FW // 2], F32, tag="ps")
                col_mm(ps_t1, [(br, [(-1, 1.0), (0, 1.0), (1, 1.0)])], 2 * h, 2 * h + 2)
                sl = slice(h * (FW // 2), (h + 1) * (FW // 2))
                nc.vector.tensor_sub(t1[:, sl], ps_t1[:], gm[:, sl])

            # ---- ring 2: out = Bcol(Brow(t1)) - t1 -----------------------
            u4 = row_u(t1, nc.gpsimd, "ub2")
            br2 = ppool.tile([P, FW], F32R, tag="br2")
            nc.vector.tensor_add(br2[:], u4[:], t1[:])
            o = opool.tile([P, FW], F32, tag="o")
            for h in range(2):
                ps_o = psum.tile([P, FW // 2], F32, tag="ps")
                col_mm(ps_o, [(br2, [(-1, 1.0), (0, 1.0), (1, 1.0)])], 2 * h, 2 * h + 2)
                sl = slice(h * (FW // 2), (h + 1) * (FW // 2))
                nc.vector.tensor_sub(o[:, sl], ps_o[:], t1[:, sl])

            nc.sync.dma_start(_plane_view(out_d[i]), o[:])

    nc.compile()
    return nc


_NC = None


def _get_nc():
    global _NC
    if _NC is None:
        _NC = _build_nc()
    return _NC


def kernel(**inputs):
    img = np.ascontiguousarray(np.asarray(inputs["img"], dtype=np.float32))
    nc = _get_nc()
    in_maps = [{"img": img[B_PER * c:B_PER * (c + 1)]} for c in range(N_CORES)]
    res = run_bass_kernel_spmd(nc, in_maps, list(range(N_CORES)))
    out = np.concatenate([res.results[c]["out"] for c in range(N_CORES)], axis=0)
    return out[:, None, :, :]



# revision 6
# speedup vs baseline: 1.7918x; 1.7918x over previous
"""Canny filter (nn_CannyFilter) Trainium2 Bass kernel.

Math (reference pipeline collapses to):
    s   = sum_c img                       (channel sum via DMA accumulate; /3 folded)
    b   = gauss3x3 (*) s / 3              (separable [g0,g1,g0] x [g0,g1,g0])
    gx  = [.5,1,.5]_col (x) [-1,0,1]_row (*) b
    gy  = [-1,0,1]_col (x) [.5,1,.5]_row (*) b
    gm  = sqrt(gx^2 + gy^2)
    t1  = ring (*) gm                     (ring = 3x3 ones minus center)
    out = ring (*) t1

Layout: one 512x512 plane = one SBUF tile [128, 2048]; partition p holds rows
4p..4p+3 (free f = c*512 + w, row = 4p + c).  W-direction (row) convs are
free-dim shifts; H-direction (col) convs: within-partition taps are free-dim
block shifts done as scaled-diagonal matmuls on PE, cross-partition taps use
shifted-diagonal lhsT matmuls.

Engine split (per plane):
 - Pool:  channel-sum DMA descriptor gen (SWDGE accumulate), u_g/u_a row sums
 - DVE:   fp16 row shifts/adds, r1 stt, ring1 hybrid assembly
 - PE:    column conv taps (fp16 lhsT x fp16 rhs -> f32 PSUM)
 - Act:   PSUM->SBUF writebacks, squares, sqrt (fp16 out)
 - SP:    output DMAs (HWDGE)

Intermediates are fp16 (DVE 2-byte 2x mode, PE 1 cycle/row); rel tol is 2e-2.
Sharding: pure data parallel, 4 images per core across 8 cores.
"""
import numpy as np
from contextlib import ExitStack

import concourse.bass as bass
import concourse.tile as tile
from concourse import bacc, mybir
from concourse.bass_utils import run_bass_kernel_spmd

N_CORES = 8
B_TOTAL = 32
B_PER = B_TOTAL // N_CORES  # 4 images per core
C, H, W = 3, 512, 512
P = 128          # SBUF partitions
RPP = H // P     # 4 rows per partition
FW = RPP * W     # 2048 free dim per plane
HF = FW // 2     # half plane free size

F32 = mybir.dt.float32
F16 = mybir.dt.float16
AF = mybir.ActivationFunctionType
OP = mybir.AluOpType

# gaussian separable vector (mu=0, sigma=1, k=3 -> exactly separable)
_g1 = np.linspace(-1.0, 1.0, 3)
_gg = np.exp(-(_g1[None, :] ** 2 + _g1[:, None] ** 2) / 2.0) / (2.0 * np.pi)
_gg = _gg / _gg.sum()
_u, _s, _vt = np.linalg.svd(_gg)
_gv = np.abs(_u[:, 0]) * np.sqrt(_s[0])
G0, G1 = float(_gv[0]), float(_gv[1])


def _plane_view(dram_ap):
    """[H, W] dram AP -> [128, 2048] (partition p = rows 4p..4p+3)."""
    return dram_ap.rearrange("(p q) w -> p (q w)", q=RPP)


def _build_nc():
    nc = bacc.Bacc("TRN2", target_bir_lowering=False, debug=False,
                   num_devices=N_CORES)
    img_d = nc.dram_tensor("img", [B_PER, C, H, W], F32, kind="ExternalInput")
    out_d = nc.dram_tensor("out", [B_PER, H, W], F32, kind="ExternalOutput")

    with tile.TileContext(nc, pool_alloc_mode="queue") as tc, ExitStack() as ctx:
        cpool = ctx.enter_context(tc.tile_pool(name="consts", bufs=1))
        spool = ctx.enter_context(tc.tile_pool(name="splanes", bufs=4))
        fpool = ctx.enter_context(tc.tile_pool(name="f16", bufs=2))
        opool = ctx.enter_context(tc.tile_pool(name="outs", bufs=2))
        psum = ctx.enter_context(tc.tile_pool(name="ps", bufs=3, space="PSUM"))
        psbnd = ctx.enter_context(tc.tile_pool(name="psb", bufs=2, space="PSUM"))

        # ---- shifted-diagonal lhsT constants (fp16) -----------------------
        # D[p, f] = f - p (int32); lhsT = (D == delta) * coef.
        # matmul semantics: out[:, f] = sum_p lhsT[p, f] * x[p]  ->
        # lhs(delta, c): out[f] = c * x[f - delta].
        dmat = cpool.tile([P, P], mybir.dt.int32)
        nc.gpsimd.iota(dmat[:], pattern=[[1, P]], base=0, channel_multiplier=-1)

        lhs_cache = {}

        def lhs(delta, coef):
            key = (delta, round(coef, 10))
            if key not in lhs_cache:
                t = cpool.tile([P, P], F16, tag=f"lhs_{len(lhs_cache)}")
                nc.vector.tensor_scalar(t[:], dmat[:], float(delta), float(coef),
                                        OP.is_equal, OP.mult)
                lhs_cache[key] = t
            return lhs_cache[key]

        # ---- input DMAs: channel-sum accumulate, plane-major --------------
        s_tiles = []
        for i in range(B_PER):
            s_i = spool.tile([P, FW], F32, tag="s")
            s_tiles.append(s_i)
        for i in range(B_PER):
            for ch in range(C):
                nc.gpsimd.dma_start(
                    s_tiles[i][:], _plane_view(img_d[i, ch]),
                    accum_op=(OP.bypass if ch == 0 else OP.add))

        # ---- helpers ------------------------------------------------------
        def v3(t):
            return t[:].rearrange("p (c w) -> p c w", w=W)

        def row3(eng, out_t, x_t, h, sub=False, edge_eng=None):
            """out = xR - xL (sub) or xL + xR (add) on blocks 2h..2h+1,
            with zero-pad edge fixups (on edge_eng, default gpsimd)."""
            ee = edge_eng or nc.gpsimd
            ov, xv = v3(out_t), v3(x_t)
            c0, c1 = 2 * h, 2 * h + 2
            if sub:
                eng.tensor_sub(ov[:, c0:c1, 1:W - 1],
                               xv[:, c0:c1, 2:W], xv[:, c0:c1, 0:W - 2])
                ee.tensor_copy(ov[:, c0:c1, 0:1], xv[:, c0:c1, 1:2])
                ee.tensor_scalar_mul(ov[:, c0:c1, W - 1:W],
                                     xv[:, c0:c1, W - 2:W - 1], -1.0)
            else:
                eng.tensor_add(ov[:, c0:c1, 1:W - 1],
                               xv[:, c0:c1, 0:W - 2], xv[:, c0:c1, 2:W])
                ee.tensor_copy(ov[:, c0:c1, 0:1], xv[:, c0:c1, 1:2])
                ee.tensor_copy(ov[:, c0:c1, W - 1:W], xv[:, c0:c1, W - 2:W - 1])

        def col_mm(ps_t, inputs, h):
            """H-direction conv taps into psum half-tile ps_t [P, HF].
            inputs: list of (tile, [(dr, coef), ...]).  Out row r = 4p + c
            taps row r + dr: cc = c + dr in 0..3 -> same partition block cc;
            cc == -1 -> lhs(+1) on block 3; cc == 4 -> lhs(-1) on block 0.
            Matmuls are ordered lhs-major (fewer ldweights) while keeping
            start/stop per psum block region correct."""
            per_block = {}
            for c in (2 * h, 2 * h + 1):
                todo = []
                for x_t, taps in inputs:
                    xv = x_t[:]
                    for dr, coef in taps:
                        cc = c + dr
                        if cc == -1:
                            todo.append((lhs(+1, coef), xv[:, 3 * W:4 * W]))
                        elif cc == RPP:
                            todo.append((lhs(-1, coef), xv[:, 0:W]))
                        else:
                            todo.append((lhs(0, coef), xv[:, cc * W:(cc + 1) * W]))
                per_block[c] = todo
            # interleave blocks, grouping by lhs tile where possible
            order = []
            for c, todo in per_block.items():
                n = len(todo)
                for j, (lt, src) in enumerate(todo):
                    order.append((id(lt), c, j, n, lt, src))
            order.sort(key=lambda e: (e[0], e[1], e[2]))
            # start/stop must follow per-block emission position, so recompute
            pos = {c: 0 for c in per_block}
            emitted = []
            for _, c, j, n, lt, src in order:
                emitted.append((c, lt, src))
            for c, lt, src in emitted:
                lc = c - 2 * h
                j = pos[c]
                n = len(per_block[c])
                nc.tensor.matmul(ps_t[:, lc * W:(lc + 1) * W], lt[:], src,
                                 start=(j == 0), stop=(j == n - 1))
                pos[c] = j + 1

        HSL = (slice(0, HF), slice(HF, FW))

        # gauss column taps on r1 (= Brow(s)/G0): fold G0 and the /3 channel
        # mean here.
        m_c = G0 * G0 / 3.0
        c_c = G1 * G0 / 3.0

        # ------------------------------------------------------------------
        # Software-pipelined emission: per-plane work is split into chunks;
        # chunks across planes are emitted sorted by estimated execution
        # time so each in-order sequencer sees its work in roughly the order
        # it becomes runnable (avoids head-of-line blocking).
        # ------------------------------------------------------------------
        state = [dict() for _ in range(B_PER)]

        def ck_a(i):  # gauss row: u_g = sL + sR ; r1 = (G1/G0) s + u_g
            s = s_tiles[i]
            u_g = fpool.tile([P, FW], F16, tag="u_g")
            for h in range(2):
                row3(nc.gpsimd, u_g, s, h)
            r1 = fpool.tile([P, FW], F16, tag="r1")
            for h in range(2):
                nc.vector.scalar_tensor_tensor(
                    r1[:, HSL[h]], s[:, HSL[h]], G1 / G0, u_g[:, HSL[h]],
                    OP.mult, OP.add)
            state[i]["r1"] = r1

        def ck_b(i):  # gauss col (PE) -> b
            r1 = state[i]["r1"]
            b = fpool.tile([P, FW], F16, tag="b")
            for h in range(2):
                ps_b = psum.tile([P, HF], F32, tag="ps")
                col_mm(ps_b, [(r1, [(-1, m_c), (0, c_c), (1, m_c)])], h)
                nc.scalar.activation(b[:, HSL[h]], ps_b[:], AF.Copy)
            state[i]["b"] = b

        def ck_c(i):  # sobel x: d = bR - bL ; q1 = (col(.5,1,.5) d)^2
            b = state[i]["b"]
            d = fpool.tile([P, FW], F16, tag="d")
            for h in range(2):
                row3(nc.vector, d, b, h, sub=True)
            q1 = fpool.tile([P, FW], F16, tag="q1")
            for h in range(2):
                ps_gx = psum.tile([P, HF], F32, tag="ps")
                col_mm(ps_gx, [(d, [(-1, 0.5), (0, 1.0), (1, 0.5)])], h)
                nc.scalar.activation(q1[:, HSL[h]], ps_gx[:], AF.Square)
            state[i]["q1"] = q1

        def ck_d(i):  # sobel y row: u_a = bL + bR
            b = state[i]["b"]
            u_a = fpool.tile([P, FW], F16, tag="u_a")
            for h in range(2):
                row3(nc.gpsimd, u_a, b, h)
            state[i]["u_a"] = u_a

        def ck_e(i):  # sobel y col on (b, u_a); gm = sqrt(q1 + q2)
            b, u_a, q1 = state[i]["b"], state[i]["u_a"], state[i]["q1"]
            q2 = fpool.tile([P, FW], F16, tag="q2")
            for h in range(2):
                ps_gy = psum.tile([P, HF], F32, tag="ps")
                col_mm(ps_gy, [(b, [(-1, -1.0), (1, 1.0)]),
                               (u_a, [(-1, -0.5), (1, 0.5)])], h)
                nc.scalar.activation(q2[:, HSL[h]], ps_gy[:], AF.Square)
            gm2 = fpool.tile([P, FW], F16, tag="gm2")
            for h in range(2):
                nc.vector.tensor_add(gm2[:, HSL[h]], q1[:, HSL[h]], q2[:, HSL[h]])
            gm = fpool.tile([P, FW], F16, tag="gm")
            for h in range(2):
                nc.scalar.activation(gm[:, HSL[h]], gm2[:, HSL[h]], AF.Sqrt)
            state[i]["gm"] = gm

        def ck_f(i):  # ring 1 (hybrid): t1 = S(gm) - gm = u5 + ucol(br)
            gm = state[i]["gm"]
            u5 = fpool.tile([P, FW], F16, tag="u5")
            for h in range(2):
                row3(nc.vector, u5, gm, h)
            br = fpool.tile([P, FW], F16, tag="br")
            for h in range(2):
                nc.vector.tensor_add(br[:, HSL[h]], u5[:, HSL[h]], gm[:, HSL[h]])
            ps_up = psbnd.tile([P, W], F32, tag="bnd")  # br[p-1, blk3] -> blk0
            nc.tensor.matmul(ps_up[:], lhs(+1, 1.0)[:], br[:, 3 * W:4 * W],
                             start=True, stop=True)
            ps_dn = psbnd.tile([P, W], F32, tag="bnd")  # br[p+1, blk0] -> blk3
            nc.tensor.matmul(ps_dn[:], lhs(-1, 1.0)[:], br[:, 0:W],
                             start=True, stop=True)
            u6 = fpool.tile([P, FW], F16, tag="u6")
            nc.vector.tensor_add(u6[:, W:3 * W], br[:, 0:2 * W], br[:, 2 * W:4 * W])
            nc.vector.tensor_add(u6[:, 0:W], br[:, W:2 * W], ps_up[:])
            nc.vector.tensor_add(u6[:, 3 * W:4 * W], br[:, 2 * W:3 * W], ps_dn[:])
            t1 = fpool.tile([P, FW], F16, tag="t1")
            for h in range(2):
                nc.vector.tensor_add(t1[:, HSL[h]], u5[:, HSL[h]], u6[:, HSL[h]])
            state[i]["t1"] = t1

        def ck_g(i):  # ring 2 (PE): out = Bcol(br2) - t1 ; write out
            t1 = state[i]["t1"]
            u7 = fpool.tile([P, FW], F16, tag="u7")
            for h in range(2):
                row3(nc.vector, u7, t1, h)
            br2 = fpool.tile([P, FW], F16, tag="br2")
            for h in range(2):
                nc.vector.tensor_add(br2[:, HSL[h]], u7[:, HSL[h]], t1[:, HSL[h]])
            o = opool.tile([P, FW], F32, tag="o")
            for h in range(2):
                ps_o = psum.tile([P, HF], F32, tag="ps")
                col_mm(ps_o, [(br2, [(-1, 1.0), (0, 1.0), (1, 1.0)]),
                              (t1, [(0, -1.0)])], h)
                nc.scalar.activation(o[:, HSL[h]], ps_o[:], AF.Copy)
            for h in range(2):
                nc.sync.dma_start(
                    _plane_view(out_d[i]).rearrange("p (h f) -> p h f", h=2)[:, h],
                    o[:, HSL[h]])

        chunks = [ck_a, ck_b, ck_c, ck_d, ck_e, ck_f, ck_g]
        # est start (us): s-ready stagger (planes arrive in pairs) + chain
        s_ready = [17.5, 20.0, 32.0, 35.0]
        dur = [2.5, 3.5, 4.0, 1.0, 4.5, 5.5, 6.0]
        sched = []
        for i in range(B_PER):
            t = s_ready[i]
            for k, ck in enumerate(chunks):
                sched.append((t, i, k))
                t += dur[k]
        sched.sort()
        for t, i, k in sched:
            chunks[k](i)

    nc.compile()
    return nc


_NC = None


def _get_nc():
    global _NC
    if _NC is None:
        _NC = _build_nc()
    return _NC


def kernel(**inputs):
    img = np.ascontiguousarray(np.asarray(inputs["img"], dtype=np.float32))
    nc = _get_nc()
    in_maps = [{"img": img[B_PER * c:B_PER * (c + 1)]} for c in range(N_CORES)]
    res = run_bass_kernel_spmd(nc, in_maps, list(range(N_CORES)))
    out = np.concatenate([res.results[c]["out"] for c in range(N_CORES)], axis=0)
    return out[:, None, :, :]


# revision 8
# speedup vs baseline: 2.0599x; 1.1496x over previous
"""Canny filter (nn_CannyFilter) Trainium2 Bass kernel.

Math (reference pipeline collapses to):
    s   = sum_c img                       (channel sum via DMA accumulate; /3 folded)
    b   = gauss3x3 (*) s / 3              (separable [g0,g1,g0] x [g0,g1,g0])
    gx  = [.5,1,.5]_col (x) [-1,0,1]_row (*) b
    gy  = [-1,0,1]_col (x) [.5,1,.5]_row (*) b
    gm  = sqrt(gx^2 + gy^2)
    t1  = ring (*) gm                     (ring = 3x3 ones minus center)
    out = ring (*) t1

Layout: one 512x512 plane = one SBUF tile [128, 2048]; partition p holds rows
4p..4p+3 (free f = c*512 + w, row = 4p + c).  W-direction (row) convs are
free-dim shifts; H-direction (col) convs: within-partition taps are free-dim
block shifts done as scaled-diagonal matmuls on PE, cross-partition taps use
shifted-diagonal lhsT matmuls.

Engine split (per plane):
 - Pool:  channel-sum DMA descriptor gen (SWDGE accumulate), u_g/u_a row sums
 - DVE:   fp16 row shifts/adds, r1 stt, ring1 hybrid assembly
 - PE:    column conv taps (fp16 lhsT x fp16 rhs -> f32 PSUM)
 - Act:   PSUM->SBUF writebacks, squares, sqrt (fp16 out)
 - SP:    output DMAs (HWDGE)

Intermediates are fp16 (DVE 2-byte 2x mode, PE 1 cycle/row); rel tol is 2e-2.
Sharding: pure data parallel, 4 images per core across 8 cores.
"""
import numpy as np
from contextlib import ExitStack

import concourse.bass as bass
import concourse.tile as tile
from concourse import bacc, mybir
from concourse.bass_utils import run_bass_kernel_spmd

N_CORES = 8
B_TOTAL = 32
B_PER = B_TOTAL // N_CORES  # 4 images per core
C, H, W = 3, 512, 512
P = 128          # SBUF partitions
RPP = H // P     # 4 rows per partition
FW = RPP * W     # 2048 free dim per plane
HF = FW // 2     # half plane free size

F32 = mybir.dt.float32
F16 = mybir.dt.float16
AF = mybir.ActivationFunctionType
OP = mybir.AluOpType

# gaussian separable vector (mu=0, sigma=1, k=3 -> exactly separable)
_g1 = np.linspace(-1.0, 1.0, 3)
_gg = np.exp(-(_g1[None, :] ** 2 + _g1[:, None] ** 2) / 2.0) / (2.0 * np.pi)
_gg = _gg / _gg.sum()
_u, _s, _vt = np.linalg.svd(_gg)
_gv = np.abs(_u[:, 0]) * np.sqrt(_s[0])
G0, G1 = float(_gv[0]), float(_gv[1])


def _plane_view(dram_ap):
    """[H, W] dram AP -> [128, 2048] (partition p = rows 4p..4p+3)."""
    return dram_ap.rearrange("(p q) w -> p (q w)", q=RPP)


def _build_nc():
    nc = bacc.Bacc("TRN2", target_bir_lowering=False, debug=False,
                   num_devices=N_CORES)
    img_d = nc.dram_tensor("img", [B_PER, C, H, W], F32, kind="ExternalInput")
    out_d = nc.dram_tensor("out", [B_PER, H, W], F32, kind="ExternalOutput")

    with tile.TileContext(nc, pool_alloc_mode="queue") as tc, ExitStack() as ctx:
        cpool = ctx.enter_context(tc.tile_pool(name="consts", bufs=1))
        spool = ctx.enter_context(tc.tile_pool(name="splanes", bufs=4))
        fpool = ctx.enter_context(tc.tile_pool(name="f16", bufs=2))
        opool = ctx.enter_context(tc.tile_pool(name="outs", bufs=2))
        psum = ctx.enter_context(tc.tile_pool(name="ps", bufs=3, space="PSUM"))
        psbnd = ctx.enter_context(tc.tile_pool(name="psb", bufs=2, space="PSUM"))

        # ---- shifted-diagonal lhsT constants (fp16) -----------------------
        # D[p, f] = f - p (int32); lhsT = (D == delta) * coef.
        # matmul semantics: out[:, f] = sum_p lhsT[p, f] * x[p]  ->
        # lhs(delta, c): out[f] = c * x[f - delta].
        dmat = cpool.tile([P, P], mybir.dt.int32)
        nc.gpsimd.iota(dmat[:], pattern=[[1, P]], base=0, channel_multiplier=-1)

        lhs_cache = {}

        def lhs(delta, coef):
            key = (delta, round(coef, 10))
            if key not in lhs_cache:
                t = cpool.tile([P, P], F16, tag=f"lhs_{len(lhs_cache)}")
                nc.vector.tensor_scalar(t[:], dmat[:], float(delta), float(coef),
                                        OP.is_equal, OP.mult)
                lhs_cache[key] = t
            return lhs_cache[key]

        # ---- input DMAs: channel-sum accumulate, plane-major --------------
        s_tiles = []
        for i in range(B_PER):
            s_i = spool.tile([P, FW], F32, tag="s")
            s_tiles.append(s_i)
        for i in range(B_PER):
            for ch in range(C):
                nc.gpsimd.dma_start(
                    s_tiles[i][:], _plane_view(img_d[i, ch]),
                    accum_op=(OP.bypass if ch == 0 else OP.add))

        # ---- helpers ------------------------------------------------------
        def v3(t):
            return t[:].rearrange("p (c w) -> p c w", w=W)

        def row3(eng, out_t, x_t, h, sub=False, edge_eng=None):
            """out = xR - xL (sub) or xL + xR (add) on blocks 2h..2h+1,
            with zero-pad edge fixups (on edge_eng, default gpsimd)."""
            ee = edge_eng or nc.gpsimd
            ov, xv = v3(out_t), v3(x_t)
            c0, c1 = 2 * h, 2 * h + 2
            if sub:
                eng.tensor_sub(ov[:, c0:c1, 1:W - 1],
                               xv[:, c0:c1, 2:W], xv[:, c0:c1, 0:W - 2])
                ee.tensor_copy(ov[:, c0:c1, 0:1], xv[:, c0:c1, 1:2])
                ee.tensor_scalar_mul(ov[:, c0:c1, W - 1:W],
                                     xv[:, c0:c1, W - 2:W - 1], -1.0)
            else:
                eng.tensor_add(ov[:, c0:c1, 1:W - 1],
                               xv[:, c0:c1, 0:W - 2], xv[:, c0:c1, 2:W])
                ee.tensor_copy(ov[:, c0:c1, 0:1], xv[:, c0:c1, 1:2])
                ee.tensor_copy(ov[:, c0:c1, W - 1:W], xv[:, c0:c1, W - 2:W - 1])

        def col_mm(ps_t, inputs, h):
            """H-direction conv taps into psum half-tile ps_t [P, HF].
            inputs: list of (tile, [(dr, coef), ...]).  Out row r = 4p + c
            taps row r + dr: cc = c + dr in 0..3 -> same partition block cc;
            cc == -1 -> lhs(+1) on block 3; cc == 4 -> lhs(-1) on block 0.
            Matmuls are ordered lhs-major (fewer ldweights) while keeping
            start/stop per psum block region correct."""
            per_block = {}
            for c in (2 * h, 2 * h + 1):
                todo = []
                for x_t, taps in inputs:
                    xv = x_t[:]
                    for dr, coef in taps:
                        cc = c + dr
                        if cc == -1:
                            todo.append((lhs(+1, coef), xv[:, 3 * W:4 * W]))
                        elif cc == RPP:
                            todo.append((lhs(-1, coef), xv[:, 0:W]))
                        else:
                            todo.append((lhs(0, coef), xv[:, cc * W:(cc + 1) * W]))
                per_block[c] = todo
            # interleave blocks, grouping by lhs tile where possible
            order = []
            for c, todo in per_block.items():
                n = len(todo)
                for j, (lt, src) in enumerate(todo):
                    order.append((id(lt), c, j, n, lt, src))
            order.sort(key=lambda e: (e[0], e[1], e[2]))
            # start/stop must follow per-block emission position, so recompute
            pos = {c: 0 for c in per_block}
            emitted = []
            for _, c, j, n, lt, src in order:
                emitted.append((c, lt, src))
            for c, lt, src in emitted:
                lc = c - 2 * h
                j = pos[c]
                n = len(per_block[c])
                nc.tensor.matmul(ps_t[:, lc * W:(lc + 1) * W], lt[:], src,
                                 start=(j == 0), stop=(j == n - 1))
                pos[c] = j + 1

        HSL = (slice(0, HF), slice(HF, FW))

        # gauss column taps on r1 (= Brow(s)/G0): fold G0 and the /3 channel
        # mean here.
        m_c = G0 * G0 / 3.0
        c_c = G1 * G0 / 3.0

        # Prebuild all lhsT constants so they are ready before first use.
        for delta, coef in [(+1, m_c), (0, m_c), (0, c_c), (-1, m_c),
                            (+1, 0.5), (0, 0.5), (0, 1.0), (-1, 0.5),
                            (+1, -1.0), (-1, 1.0), (0, -1.0),
                            (+1, -0.5), (-1, 0.5), (0, -0.5),
                            (+1, 1.0), (-1, -0.5), (0, -1.0), (0, 1.0)]:
            lhs(delta, coef)

        # ------------------------------------------------------------------
        # Software-pipelined emission: per-plane work is split into chunks;
        # chunks across planes are emitted sorted by estimated execution
        # time so each in-order sequencer sees its work in roughly the order
        # it becomes runnable (avoids head-of-line blocking).
        # ------------------------------------------------------------------
        state = [dict() for _ in range(B_PER)]

        def ck_a(i):  # gauss row: u_g = sL + sR ; r1 = (G1/G0) s + u_g
            s = s_tiles[i]
            eng = nc.vector if i < 2 else nc.gpsimd
            eeng = nc.vector if i < 2 else None
            u_g = fpool.tile([P, FW], F16, tag="u_g")
            for h in range(2):
                row3(eng, u_g, s, h, edge_eng=eeng)
            r1 = fpool.tile([P, FW], F16, tag="r1")
            for h in range(2):
                nc.vector.scalar_tensor_tensor(
                    r1[:, HSL[h]], s[:, HSL[h]], G1 / G0, u_g[:, HSL[h]],
                    OP.mult, OP.add)
            state[i]["r1"] = r1

        def ck_b(i):  # gauss col (PE) -> b
            r1 = state[i]["r1"]
            b = fpool.tile([P, FW], F16, tag="b")
            for h in range(2):
                ps_b = psum.tile([P, HF], F32, tag="ps")
                col_mm(ps_b, [(r1, [(-1, m_c), (0, c_c), (1, m_c)])], h)
                nc.scalar.activation(b[:, HSL[h]], ps_b[:], AF.Copy)
            state[i]["b"] = b

        def ck_c(i):  # sobel x: d = bR - bL ; q1 = (col(.5,1,.5) d)^2
            b = state[i]["b"]
            d = fpool.tile([P, FW], F16, tag="d")
            for h in range(2):
                row3(nc.vector, d, b, h, sub=True)
            q1 = fpool.tile([P, FW], F16, tag="q1")
            for h in range(2):
                ps_gx = psum.tile([P, HF], F32, tag="ps")
                col_mm(ps_gx, [(d, [(-1, 0.5), (0, 1.0), (1, 0.5)])], h)
                nc.scalar.activation(q1[:, HSL[h]], ps_gx[:], AF.Square)
            state[i]["q1"] = q1

        def ck_d(i):  # sobel y row: u_a = bL + bR
            b = state[i]["b"]
            eng = nc.vector if i < 2 else nc.gpsimd
            eeng = nc.vector if i < 2 else None
            u_a = fpool.tile([P, FW], F16, tag="u_a")
            for h in range(2):
                row3(eng, u_a, b, h, edge_eng=eeng)
            state[i]["u_a"] = u_a

        def ck_e(i):  # sobel y col on (b, u_a); gm = sqrt(q1 + q2)
            b, u_a, q1 = state[i]["b"], state[i]["u_a"], state[i]["q1"]
            q2 = fpool.tile([P, FW], F16, tag="q2")
            for h in range(2):
                ps_gy = psum.tile([P, HF], F32, tag="ps")
                col_mm(ps_gy, [(b, [(-1, -1.0), (1, 1.0)]),
                               (u_a, [(-1, -0.5), (1, 0.5)])], h)
                nc.scalar.activation(q2[:, HSL[h]], ps_gy[:], AF.Square)
            gm2 = fpool.tile([P, FW], F16, tag="gm2")
            for h in range(2):
                nc.vector.tensor_add(gm2[:, HSL[h]], q1[:, HSL[h]], q2[:, HSL[h]])
            gm = fpool.tile([P, FW], F16, tag="gm")
            for h in range(2):
                nc.scalar.activation(gm[:, HSL[h]], gm2[:, HSL[h]], AF.Sqrt)
            state[i]["gm"] = gm

        def ck_f(i):  # ring 1 (hybrid): t1 = S(gm) - gm = u5 + ucol(br)
            gm = state[i]["gm"]
            u5 = fpool.tile([P, FW], F16, tag="u5")
            for h in range(2):
                row3(nc.vector, u5, gm, h)
            br = fpool.tile([P, FW], F16, tag="br")
            for h in range(2):
                nc.vector.tensor_add(br[:, HSL[h]], u5[:, HSL[h]], gm[:, HSL[h]])
            ps_up = psbnd.tile([P, W], F32, tag="bnd")  # br[p-1, blk3] -> blk0
            nc.tensor.matmul(ps_up[:], lhs(+1, 1.0)[:], br[:, 3 * W:4 * W],
                             start=True, stop=True)
            ps_dn = psbnd.tile([P, W], F32, tag="bnd")  # br[p+1, blk0] -> blk3
            nc.tensor.matmul(ps_dn[:], lhs(-1, 1.0)[:], br[:, 0:W],
                             start=True, stop=True)
            u6 = fpool.tile([P, FW], F16, tag="u6")
            nc.vector.tensor_add(u6[:, W:3 * W], br[:, 0:2 * W], br[:, 2 * W:4 * W])
            nc.vector.tensor_add(u6[:, 0:W], br[:, W:2 * W], ps_up[:])
            nc.vector.tensor_add(u6[:, 3 * W:4 * W], br[:, 2 * W:3 * W], ps_dn[:])
            t1 = fpool.tile([P, FW], F16, tag="t1")
            for h in range(2):
                nc.vector.tensor_add(t1[:, HSL[h]], u5[:, HSL[h]], u6[:, HSL[h]])
            state[i]["t1"] = t1

        def ck_g(i):  # ring 2 (PE): out = Bcol(br2) - t1 ; write out
            t1 = state[i]["t1"]
            u7 = fpool.tile([P, FW], F16, tag="u7")
            for h in range(2):
                row3(nc.vector, u7, t1, h)
            br2 = fpool.tile([P, FW], F16, tag="br2")
            for h in range(2):
                nc.vector.tensor_add(br2[:, HSL[h]], u7[:, HSL[h]], t1[:, HSL[h]])
            o = opool.tile([P, FW], F32, tag="o")
            for h in range(2):
                ps_o = psum.tile([P, HF], F32, tag="ps")
                col_mm(ps_o, [(br2, [(-1, 1.0), (0, 1.0), (1, 1.0)]),
                              (t1, [(0, -1.0)])], h)
                nc.scalar.activation(o[:, HSL[h]], ps_o[:], AF.Copy)
            for h in range(2):
                nc.sync.dma_start(
                    _plane_view(out_d[i]).rearrange("p (h f) -> p h f", h=2)[:, h],
                    o[:, HSL[h]])

        chunks = [ck_a, ck_b, ck_c, ck_d, ck_e, ck_f, ck_g]
        # est start (us): s-ready stagger (planes arrive in pairs) + chain
        s_ready = [17.5, 20.0, 32.0, 35.0]
        dur = [2.0, 3.0, 3.5, 1.0, 4.0, 5.0, 5.5]
        sched = []
        for i in range(B_PER):
            t = s_ready[i]
            for k, ck in enumerate(chunks):
                sched.append((t, i, k))
                t += dur[k]
        sched.sort()
        for t, i, k in sched:
            chunks[k](i)

    nc.compile()
    return nc


_NC = None


def _get_nc():
    global _NC
    if _NC is None:
        _NC = _build_nc()
    return _NC


def kernel(**inputs):
    img = np.ascontiguousarray(np.asarray(inputs["img"], dtype=np.float32))
    nc = _get_nc()
    in_maps = [{"img": img[B_PER * c:B_PER * (c + 1)]} for c in range(N_CORES)]
    res = run_bass_kernel_spmd(nc, in_maps, list(range(N_CORES)))
    out = np.concatenate([res.results[c]["out"] for c in range(N_CORES)], axis=0)
    return out[:, None, :, :]


# revision 23
# speedup vs baseline: 2.4915x; 1.2095x over previous
"""Canny filter (nn_CannyFilter) Trainium2 Bass kernel.

Math (reference pipeline collapses to):
    s   = sum_c img                       (channel sum via DMA accumulate; /3 folded)
    b   = gauss3x3 (*) s / 3              (separable [g0,g1,g0] x [g0,g1,g0])
    gx  = [.5,1,.5]_col (x) [-1,0,1]_row (*) b
    gy  = [-1,0,1]_col (x) [.5,1,.5]_row (*) b
    gm  = sqrt(gx^2 + gy^2)
    t1  = ring (*) gm                     (ring = 3x3 ones minus center)
    out = ring (*) t1

Layout: one 512x512 plane = one SBUF tile [128, 2048]; partition p holds rows
4p..4p+3 (free f = c*512 + w, row = 4p + c).  W-direction (row) convs are
free-dim shifts; H-direction (col) convs: within-partition taps are free-dim
block shifts done as scaled-diagonal matmuls on PE, cross-partition taps use
shifted-diagonal lhsT matmuls.

Engine split (per plane):
 - Pool:  channel-sum DMA descriptor gen (SWDGE accumulate), u_g/u_a row sums
 - DVE:   fp16 row shifts/adds, r1 stt, ring1 hybrid assembly
 - PE:    column conv taps (fp16 lhsT x fp16 rhs -> f32 PSUM)
 - Act:   PSUM->SBUF writebacks, squares, sqrt (fp16 out)
 - SP:    output DMAs (HWDGE)

Intermediates are fp16 (DVE 2-byte 2x mode, PE 1 cycle/row); rel tol is 2e-2.
Sharding: pure data parallel, 4 images per core across 8 cores.
"""
import numpy as np
from contextlib import ExitStack

import concourse.bass as bass
import concourse.tile as tile
from concourse import bacc, mybir
from concourse.bass_utils import run_bass_kernel_spmd

N_CORES = 8
B_TOTAL = 32
B_PER = B_TOTAL // N_CORES  # 4 images per core
C, H, W = 3, 512, 512
P = 128          # SBUF partitions
RPP = H // P     # 4 rows per partition
FW = RPP * W     # 2048 free dim per plane
HF = FW // 2     # half plane free size

F32 = mybir.dt.float32
F16 = mybir.dt.float16
AF = mybir.ActivationFunctionType
OP = mybir.AluOpType

# per-plane implementation choices (tuned via TimelineSim sweep)
CONFIG = {
    "ring1": ["pair", "pair", "pair", "pair"],
    "ring2": ["pair_dve", "pair_dve", "pair_dve", "pair_dve"],
    "sp_c0": 0,   # planes whose channel-0 load goes via SP HWDGE
    "sy": "pair",     # "pair" (4 PE taps) or "expl" (a-stt on DVE, 2 taps)
    "gauss": "full",  # "full" (3 taps + Act writeback) or "pd" (2 taps + DVE stt)
    "sx": "full",     # "full" (3 taps + Act square) or "pd" (2 taps + DVE comb)
    "half_dma": 0,    # split input channel DMAs into half-planes
    "wide_mm": 0,     # merge per-block matmuls into [128,1024] where possible
}

# gaussian separable vector (mu=0, sigma=1, k=3 -> exactly separable)
_g1 = np.linspace(-1.0, 1.0, 3)
_gg = np.exp(-(_g1[None, :] ** 2 + _g1[:, None] ** 2) / 2.0) / (2.0 * np.pi)
_gg = _gg / _gg.sum()
_u, _s, _vt = np.linalg.svd(_gg)
_gv = np.abs(_u[:, 0]) * np.sqrt(_s[0])
G0, G1 = float(_gv[0]), float(_gv[1])


def _plane_view(dram_ap):
    """[H, W] dram AP -> [128, 2048] (partition p = rows 4p..4p+3)."""
    return dram_ap.rearrange("(p q) w -> p (q w)", q=RPP)


def _build_nc():
    nc = bacc.Bacc("TRN2", target_bir_lowering=False, debug=False,
                   num_devices=N_CORES)
    img_d = nc.dram_tensor("img", [B_PER, C, H, W], F32, kind="ExternalInput")
    out_d = nc.dram_tensor("out", [B_PER, H, W], F32, kind="ExternalOutput")

    with tile.TileContext(nc, pool_alloc_mode="queue") as tc, ExitStack() as ctx:
        cpool = ctx.enter_context(tc.tile_pool(name="consts", bufs=1))
        spool = ctx.enter_context(tc.tile_pool(name="splanes", bufs=4))
        fpool = ctx.enter_context(tc.tile_pool(name="f16", bufs=2))
        opool = ctx.enter_context(tc.tile_pool(name="outs", bufs=2))
        psum = ctx.enter_context(tc.tile_pool(name="ps", bufs=3, space="PSUM"))
        psbnd = ctx.enter_context(tc.tile_pool(name="psb", bufs=2, space="PSUM"))

        # ---- shifted-diagonal lhsT constants (fp16) -----------------------
        # D[p, f] = f - p (int32); lhsT = (D == delta) * coef.
        # matmul semantics: out[:, f] = sum_p lhsT[p, f] * x[p]  ->
        # lhs(delta, c): out[f] = c * x[f - delta].
        dmat = cpool.tile([P, P], mybir.dt.int32)
        nc.gpsimd.iota(dmat[:], pattern=[[1, P]], base=0, channel_multiplier=-1)

        lhs_cache = {}

        def lhs(delta, coef):
            key = (delta, round(coef, 10))
            if key not in lhs_cache:
                t = cpool.tile([P, P], F16, tag=f"lhs_{len(lhs_cache)}")
                nc.vector.tensor_scalar(t[:], dmat[:], float(delta), float(coef),
                                        OP.is_equal, OP.mult)
                lhs_cache[key] = t
            return lhs_cache[key]

        HSL0 = (slice(0, HF), slice(HF, FW))
        # ---- input DMAs: channel-sum accumulate, plane-major --------------
        s_tiles = []
        for i in range(B_PER):
            s_i = spool.tile([P, FW], F32, tag="s")
            s_tiles.append(s_i)
        for i in range(B_PER):
            for ch in range(C):
                op = OP.bypass if ch == 0 else OP.add
                if ch == 0 and i < CONFIG["sp_c0"]:
                    nc.sync.dma_start(s_tiles[i][:], _plane_view(img_d[i, ch]))
                elif CONFIG["half_dma"]:
                    pv = _plane_view(img_d[i, ch]).rearrange(
                        "p (h f) -> p h f", h=2)
                    for h in range(2):
                        nc.gpsimd.dma_start(s_tiles[i][:, HSL0[h]], pv[:, h],
                                            accum_op=op)
                else:
                    nc.gpsimd.dma_start(
                        s_tiles[i][:], _plane_view(img_d[i, ch]),
                        accum_op=op)

        # ---- helpers ------------------------------------------------------
        def v3(t):
            return t[:].rearrange("p (c w) -> p c w", w=W)

        def row3(eng, out_t, x_t, h, sub=False, edge_eng=None):
            """out = xR - xL (sub) or xL + xR (add) on blocks 2h..2h+1,
            with zero-pad edge fixups (on edge_eng, default gpsimd)."""
            ee = edge_eng or nc.gpsimd
            ov, xv = v3(out_t), v3(x_t)
            c0, c1 = 2 * h, 2 * h + 2
            if sub:
                eng.tensor_sub(ov[:, c0:c1, 1:W - 1],
                               xv[:, c0:c1, 2:W], xv[:, c0:c1, 0:W - 2])
                ee.tensor_copy(ov[:, c0:c1, 0:1], xv[:, c0:c1, 1:2])
                ee.tensor_scalar_mul(ov[:, c0:c1, W - 1:W],
                                     xv[:, c0:c1, W - 2:W - 1], -1.0)
            else:
                eng.tensor_add(ov[:, c0:c1, 1:W - 1],
                               xv[:, c0:c1, 0:W - 2], xv[:, c0:c1, 2:W])
                ee.tensor_copy(ov[:, c0:c1, 0:1], xv[:, c0:c1, 1:2])
                ee.tensor_copy(ov[:, c0:c1, W - 1:W], xv[:, c0:c1, W - 2:W - 1])

        def col_mm(ps_t, inputs, h):
            """H-direction conv taps into psum half-tile ps_t [P, HF].
            inputs: list of (tile, [(dr, coef), ...]).  Out row r = 4p + c
            taps row r + dr: cc = c + dr in 0..3 -> same partition block cc;
            cc == -1 -> lhs(+1) on block 3; cc == 4 -> lhs(-1) on block 0.
            Matmuls are ordered lhs-major (fewer ldweights) while keeping
            start/stop per psum block region correct."""
            c0, c1 = 2 * h, 2 * h + 1
            wides, narrows = [], []  # (lhs, src_ap, regions, out_slice)
            for x_t, taps in inputs:
                xv = x_t[:]
                for dr, coef in taps:
                    cca, ccb = c0 + dr, c1 + dr
                    if CONFIG["wide_mm"] and 0 <= cca and ccb <= RPP - 1:
                        wides.append((lhs(0, coef), xv[:, cca * W:(ccb + 1) * W],
                                      (0, 1), slice(0, 2 * W)))
                        continue
                    for lc, cc in ((0, cca), (1, ccb)):
                        osl = slice(lc * W, (lc + 1) * W)
                        if cc == -1:
                            narrows.append((lhs(+1, coef), xv[:, 3 * W:4 * W],
                                            (lc,), osl))
                        elif cc == RPP:
                            narrows.append((lhs(-1, coef), xv[:, 0:W],
                                            (lc,), osl))
                        else:
                            narrows.append((lhs(0, coef),
                                            xv[:, cc * W:(cc + 1) * W],
                                            (lc,), osl))
            narrows.sort(key=lambda e: (id(e[0]), e[2]))
            todo = wides + narrows
            first = {}; last = {}
            for j, (lt, src, regs, osl) in enumerate(todo):
                for r in regs:
                    first.setdefault(r, j)
                    last[r] = j
            for j, (lt, src, regs, osl) in enumerate(todo):
                st = all(first[r] == j for r in regs)
                sp = all(last[r] == j for r in regs)
                # every region's first writer must carry start; verify
                assert all((first[r] == j) == st for r in regs)
                assert all((last[r] == j) == sp for r in regs)
                nc.tensor.matmul(ps_t[:, osl], lt[:], src, start=st, stop=sp)

        HSL = (slice(0, HF), slice(HF, FW))

        # gauss column taps on r1 (= Brow(s)/G0): fold G0 and the /3 channel
        # mean here.
        m_c = G0 * G0 / 3.0
        c_c = G1 * G0 / 3.0

        # Prebuild all lhsT constants so they are ready before first use.
        for delta, coef in [(+1, m_c), (0, m_c), (0, c_c), (-1, m_c),
                            (+1, 0.5), (0, 0.5), (0, 1.0), (-1, 0.5),
                            (+1, -1.0), (-1, 1.0), (0, -1.0),
                            (+1, -0.5), (-1, 0.5), (0, -0.5),
                            (+1, 1.0), (-1, -0.5), (0, -1.0), (0, 1.0)]:
            lhs(delta, coef)

        # ------------------------------------------------------------------
        # Software-pipelined emission: per-plane work is split into chunks;
        # chunks across planes are emitted sorted by estimated execution
        # time so each in-order sequencer sees its work in roughly the order
        # it becomes runnable (avoids head-of-line blocking).
        # ------------------------------------------------------------------
        state = [dict() for _ in range(B_PER)]

        def ck_a(i):  # gauss row: u_g = sL + sR ; r1 = (G1/G0) s + u_g
            s = s_tiles[i]
            eng = nc.vector if i < 2 else nc.gpsimd
            eeng = nc.vector if i < 2 else None
            u_g = fpool.tile([P, FW], F16, tag="u_g")
            for h in range(2):
                row3(eng, u_g, s, h, edge_eng=eeng)
            r1 = fpool.tile([P, FW], F16, tag="r1")
            for h in range(2):
                nc.vector.scalar_tensor_tensor(
                    r1[:, HSL[h]], s[:, HSL[h]], G1 / G0, u_g[:, HSL[h]],
                    OP.mult, OP.add)
            state[i]["r1"] = r1

        def ck_b(i):  # gauss col (PE) -> b
            r1 = state[i]["r1"]
            b = fpool.tile([P, FW], F16, tag="b")
            for h in range(2):
                ps_b = psum.tile([P, HF], F32, tag="ps")
                if CONFIG["gauss"] == "pd":
                    # psum = m_c * ucol(r1); b = c_c*r1 + psum on DVE
                    col_mm(ps_b, [(r1, [(-1, m_c), (1, m_c)])], h)
                    nc.vector.scalar_tensor_tensor(
                        b[:, HSL[h]], r1[:, HSL[h]], c_c, ps_b[:],
                        OP.mult, OP.add)
                else:
                    col_mm(ps_b, [(r1, [(-1, m_c), (0, c_c), (1, m_c)])], h)
                    nc.scalar.activation(b[:, HSL[h]], ps_b[:], AF.Copy)
            state[i]["b"] = b

        def ck_c(i):  # sobel x: d = bR - bL ; q1 = (col(.5,1,.5) d)^2
            b = state[i]["b"]
            d = fpool.tile([P, FW], F16, tag="d")
            for h in range(2):
                row3(nc.vector, d, b, h, sub=True)
            q1 = fpool.tile([P, FW], F16, tag="q1")
            if CONFIG["sx"] == "pd":
                gx = fpool.tile([P, FW], F16, tag="gx")
                for h in range(2):
                    ps_gx = psum.tile([P, HF], F32, tag="ps")
                    col_mm(ps_gx, [(d, [(-1, 0.5), (1, 0.5)])], h)
                    nc.vector.tensor_add(gx[:, HSL[h]], d[:, HSL[h]], ps_gx[:])
                for h in range(2):
                    nc.scalar.activation(q1[:, HSL[h]], gx[:, HSL[h]], AF.Square)
            else:
                for h in range(2):
                    ps_gx = psum.tile([P, HF], F32, tag="ps")
                    col_mm(ps_gx, [(d, [(-1, 0.5), (0, 1.0), (1, 0.5)])], h)
                    nc.scalar.activation(q1[:, HSL[h]], ps_gx[:], AF.Square)
            state[i]["q1"] = q1

        def ck_d(i):  # sobel y row: u_a = bL + bR
            b = state[i]["b"]
            eng = nc.vector if i < 2 else nc.gpsimd
            eeng = nc.vector if i < 2 else None
            u_a = fpool.tile([P, FW], F16, tag="u_a")
            for h in range(2):
                row3(eng, u_a, b, h, edge_eng=eeng)
            state[i]["u_a"] = u_a

        def ck_e(i):  # sobel y col on (b, u_a); gm = sqrt(q1 + q2)
            b, u_a, q1 = state[i]["b"], state[i]["u_a"], state[i]["q1"]
            q2 = fpool.tile([P, FW], F16, tag="q2")
            if CONFIG["sy"] == "expl":
                # a = 2b + u_a (DVE stt); gy = 0.5*(a[r+1]-a[r-1]): 2 PE taps
                a = fpool.tile([P, FW], F16, tag="a")
                for h in range(2):
                    nc.vector.scalar_tensor_tensor(
                        a[:, HSL[h]], b[:, HSL[h]], 2.0, u_a[:, HSL[h]],
                        OP.mult, OP.add)
                sy_inputs = [(a, [(-1, -0.5), (1, 0.5)])]
            else:
                sy_inputs = [(b, [(-1, -1.0), (1, 1.0)]),
                             (u_a, [(-1, -0.5), (1, 0.5)])]
            for h in range(2):
                ps_gy = psum.tile([P, HF], F32, tag="ps")
                col_mm(ps_gy, sy_inputs, h)
                nc.scalar.activation(q2[:, HSL[h]], ps_gy[:], AF.Square)
            gm2 = fpool.tile([P, FW], F16, tag="gm2")
            for h in range(2):
                nc.vector.tensor_add(gm2[:, HSL[h]], q1[:, HSL[h]], q2[:, HSL[h]])
            gm = fpool.tile([P, FW], F16, tag="gm")
            for h in range(2):
                nc.scalar.activation(gm[:, HSL[h]], gm2[:, HSL[h]], AF.Sqrt)
            state[i]["gm"] = gm

        def ck_f(i):  # ring 1: t1 = S(gm) - gm
            gm = state[i]["gm"]
            u5 = fpool.tile([P, FW], F16, tag="u5")
            for h in range(2):
                row3(nc.vector, u5, gm, h)
            if CONFIG["ring1"][i] == "pair":
                # t1 = Bcol(u5) + ucol(gm): 5 PE taps, Act writeback
                t1 = fpool.tile([P, FW], F16, tag="t1")
                for h in range(2):
                    ps_t = psum.tile([P, HF], F32, tag="ps")
                    col_mm(ps_t, [(u5, [(-1, 1.0), (0, 1.0), (1, 1.0)]),
                                  (gm, [(-1, 1.0), (1, 1.0)])], h)
                    nc.scalar.activation(t1[:, HSL[h]], ps_t[:], AF.Copy)
            elif CONFIG["ring1"][i] == "pair_dve":
                # psum = ucol(u5) + ucol(gm) (4 taps); t1 = u5 + psum on DVE
                t1 = fpool.tile([P, FW], F16, tag="t1")
                for h in range(2):
                    ps_t = psum.tile([P, HF], F32, tag="ps")
                    col_mm(ps_t, [(u5, [(-1, 1.0), (1, 1.0)]),
                                  (gm, [(-1, 1.0), (1, 1.0)])], h)
                    nc.vector.tensor_add(t1[:, HSL[h]], u5[:, HSL[h]], ps_t[:])
            else:  # hyb: t1 = u5 + ucol(br), br = u5 + gm
                br = fpool.tile([P, FW], F16, tag="br")
                for h in range(2):
                    nc.vector.tensor_add(br[:, HSL[h]], u5[:, HSL[h]],
                                         gm[:, HSL[h]])
                ps_up = psbnd.tile([P, W], F32, tag="bnd")  # br[p-1,blk3]->blk0
                nc.tensor.matmul(ps_up[:], lhs(+1, 1.0)[:], br[:, 3 * W:4 * W],
                                 start=True, stop=True)
                ps_dn = psbnd.tile([P, W], F32, tag="bnd")  # br[p+1,blk0]->blk3
                nc.tensor.matmul(ps_dn[:], lhs(-1, 1.0)[:], br[:, 0:W],
                                 start=True, stop=True)
                u6 = fpool.tile([P, FW], F16, tag="u6")
                nc.vector.tensor_add(u6[:, W:3 * W], br[:, 0:2 * W],
                                     br[:, 2 * W:4 * W])
                nc.vector.tensor_add(u6[:, 0:W], br[:, W:2 * W], ps_up[:])
                nc.vector.tensor_add(u6[:, 3 * W:4 * W], br[:, 2 * W:3 * W],
                                     ps_dn[:])
                t1 = fpool.tile([P, FW], F16, tag="t1")
                for h in range(2):
                    nc.vector.tensor_add(t1[:, HSL[h]], u5[:, HSL[h]],
                                         u6[:, HSL[h]])
            state[i]["t1"] = t1

        def ck_g(i):  # ring 2 (PE): out = S(t1) - t1 ; write out
            t1 = state[i]["t1"]
            u7 = fpool.tile([P, FW], F16, tag="u7")
            for h in range(2):
                row3(nc.vector, u7, t1, h)
            o = opool.tile([P, FW], F32, tag="o")
            if CONFIG["ring2"][i] == "pair_dve":
                # psum = ucol(u7) + ucol(t1) (4 taps); o = u7 + psum on DVE
                for h in range(2):
                    ps_o = psum.tile([P, HF], F32, tag="ps")
                    col_mm(ps_o, [(u7, [(-1, 1.0), (1, 1.0)]),
                                  (t1, [(-1, 1.0), (1, 1.0)])], h)
                    nc.vector.tensor_add(o[:, HSL[h]], u7[:, HSL[h]], ps_o[:])
            else:
                if CONFIG["ring2"][i] == "pair":
                    inputs = [(u7, [(-1, 1.0), (0, 1.0), (1, 1.0)]),
                              (t1, [(-1, 1.0), (1, 1.0)])]
                else:  # expl: br2 = u7 + t1
                    br2 = fpool.tile([P, FW], F16, tag="br2")
                    for h in range(2):
                        nc.vector.tensor_add(br2[:, HSL[h]], u7[:, HSL[h]],
                                             t1[:, HSL[h]])
                    inputs = [(br2, [(-1, 1.0), (0, 1.0), (1, 1.0)]),
                              (t1, [(0, -1.0)])]
                for h in range(2):
                    ps_o = psum.tile([P, HF], F32, tag="ps")
                    col_mm(ps_o, inputs, h)
                    nc.scalar.activation(o[:, HSL[h]], ps_o[:], AF.Copy)
            for h in range(2):
                nc.sync.dma_start(
                    _plane_view(out_d[i]).rearrange("p (h f) -> p h f", h=2)[:, h],
                    o[:, HSL[h]])

        chunks = [ck_a, ck_b, ck_c, ck_d, ck_e, ck_f, ck_g]
        # est start (us): s-ready stagger (planes arrive in pairs) + chain
        s_ready = CONFIG.get("s_ready", [17.5, 20.0, 32.0, 35.0])
        dur = CONFIG.get("dur", [2.0, 3.0, 3.5, 1.0, 4.0, 5.0, 5.5])
        sched = []
        for i in range(B_PER):
            t = s_ready[i]
            for k, ck in enumerate(chunks):
                sched.append((t, i, k))
                t += dur[k]
        sched.sort()
        for t, i, k in sched:
            chunks[k](i)

    nc.compile()
    return nc


_NC = None


def _get_nc():
    global _NC
    if _NC is None:
        _NC = _build_nc()
    return _NC


def kernel(**inputs):
    img = np.ascontiguousarray(np.asarray(inputs["img"], dtype=np.float32))
    nc = _get_nc()
    in_maps = [{"img": img[B_PER * c:B_PER * (c + 1)]} for c in range(N_CORES)]
    res = run_bass_kernel_spmd(nc, in_maps, list(range(N_CORES)))
    out = np.concatenate([res.results[c]["out"] for c in range(N_CORES)], axis=0)
    return out[:, None, :, :]


# revision 25
# speedup vs baseline: 2.5107x; 1.0077x over previous
"""Canny filter (nn_CannyFilter) Trainium2 Bass kernel.

Math (reference pipeline collapses to):
    s   = sum_c img                       (channel sum via DMA accumulate; /3 folded)
    b   = gauss3x3 (*) s / 3              (separable [g0,g1,g0] x [g0,g1,g0])
    gx  = [.5,1,.5]_col (x) [-1,0,1]_row (*) b
    gy  = [-1,0,1]_col (x) [.5,1,.5]_row (*) b
    gm  = sqrt(gx^2 + gy^2)
    t1  = ring (*) gm                     (ring = 3x3 ones minus center)
    out = ring (*) t1

Layout: one 512x512 plane = one SBUF tile [128, 2048]; partition p holds rows
4p..4p+3 (free f = c*512 + w, row = 4p + c).  W-direction (row) convs are
free-dim shifts; H-direction (col) convs: within-partition taps are free-dim
block shifts done as scaled-diagonal matmuls on PE, cross-partition taps use
shifted-diagonal lhsT matmuls.

Engine split (per plane):
 - Pool:  channel-sum DMA descriptor gen (SWDGE accumulate), u_g/u_a row sums
 - DVE:   fp16 row shifts/adds, r1 stt, ring1 hybrid assembly
 - PE:    column conv taps (fp16 lhsT x fp16 rhs -> f32 PSUM)
 - Act:   PSUM->SBUF writebacks, squares, sqrt (fp16 out)
 - SP:    output DMAs (HWDGE)

Intermediates are fp16 (DVE 2-byte 2x mode, PE 1 cycle/row); rel tol is 2e-2.
Sharding: pure data parallel, 4 images per core across 8 cores.
"""
import numpy as np
from contextlib import ExitStack

import concourse.bass as bass
import concourse.tile as tile
from concourse import bacc, mybir
from concourse.bass_utils import run_bass_kernel_spmd

N_CORES = 8
B_TOTAL = 32
B_PER = B_TOTAL // N_CORES  # 4 images per core
C, H, W = 3, 512, 512
P = 128          # SBUF partitions
RPP = H // P     # 4 rows per partition
FW = RPP * W     # 2048 free dim per plane
HF = FW // 2     # half plane free size

F32 = mybir.dt.float32
F16 = mybir.dt.float16
AF = mybir.ActivationFunctionType
OP = mybir.AluOpType

# per-plane implementation choices (tuned via TimelineSim sweep)
CONFIG = {
    "ring1": ["pair", "pair", "pair", "pair"],
    "ring2": ["pair_dve", "pair_dve", "pair_dve", "pair_dve"],
    "sp_c0": 0,   # planes whose channel-0 load goes via SP HWDGE
    "sy": "pair",     # "pair" (4 PE taps) or "expl" (a-stt on DVE, 2 taps)
    "gauss": "full",  # "full" (3 taps + Act writeback) or "pd" (2 taps + DVE stt)
    "sx": "full",     # "full" (3 taps + Act square) or "pd" (2 taps + DVE comb)
    "half_dma": 0,    # split input channel DMAs into half-planes
    "wide_mm": 0,     # merge per-block matmuls into [128,1024] where possible
    "ps_bufs": 4,     # big-psum ring depth (8 banks when psbnd unused)
    "dur": [2.6, 3.9, 4.6, 1.3, 5.2, 6.5, 7.2],  # chunk time estimates (us)
}

# gaussian separable vector (mu=0, sigma=1, k=3 -> exactly separable)
_g1 = np.linspace(-1.0, 1.0, 3)
_gg = np.exp(-(_g1[None, :] ** 2 + _g1[:, None] ** 2) / 2.0) / (2.0 * np.pi)
_gg = _gg / _gg.sum()
_u, _s, _vt = np.linalg.svd(_gg)
_gv = np.abs(_u[:, 0]) * np.sqrt(_s[0])
G0, G1 = float(_gv[0]), float(_gv[1])


def _plane_view(dram_ap):
    """[H, W] dram AP -> [128, 2048] (partition p = rows 4p..4p+3)."""
    return dram_ap.rearrange("(p q) w -> p (q w)", q=RPP)


def _build_nc():
    nc = bacc.Bacc("TRN2", target_bir_lowering=False, debug=False,
                   num_devices=N_CORES)
    img_d = nc.dram_tensor("img", [B_PER, C, H, W], F32, kind="ExternalInput")
    out_d = nc.dram_tensor("out", [B_PER, H, W], F32, kind="ExternalOutput")

    with tile.TileContext(nc, pool_alloc_mode="queue") as tc, ExitStack() as ctx:
        cpool = ctx.enter_context(tc.tile_pool(name="consts", bufs=1))
        spool = ctx.enter_context(tc.tile_pool(name="splanes", bufs=4))
        fpool = ctx.enter_context(tc.tile_pool(name="f16", bufs=2))
        opool = ctx.enter_context(tc.tile_pool(name="outs", bufs=2))
        psum = ctx.enter_context(tc.tile_pool(
            name="ps", bufs=CONFIG.get("ps_bufs", 3), space="PSUM"))
        psbnd = None
        if any(r == "hyb" for r in CONFIG["ring1"]):
            psbnd = ctx.enter_context(
                tc.tile_pool(name="psb", bufs=2, space="PSUM"))

        # ---- shifted-diagonal lhsT constants (fp16) -----------------------
        # D[p, f] = f - p (int32); lhsT = (D == delta) * coef.
        # matmul semantics: out[:, f] = sum_p lhsT[p, f] * x[p]  ->
        # lhs(delta, c): out[f] = c * x[f - delta].
        dmat = cpool.tile([P, P], mybir.dt.int32)
        nc.gpsimd.iota(dmat[:], pattern=[[1, P]], base=0, channel_multiplier=-1)

        lhs_cache = {}

        def lhs(delta, coef):
            key = (delta, round(coef, 10))
            if key not in lhs_cache:
                t = cpool.tile([P, P], F16, tag=f"lhs_{len(lhs_cache)}")
                nc.vector.tensor_scalar(t[:], dmat[:], float(delta), float(coef),
                                        OP.is_equal, OP.mult)
                lhs_cache[key] = t
            return lhs_cache[key]

        HSL0 = (slice(0, HF), slice(HF, FW))
        # ---- input DMAs: channel-sum accumulate, plane-major --------------
        s_tiles = []
        for i in range(B_PER):
            s_i = spool.tile([P, FW], F32, tag="s")
            s_tiles.append(s_i)
        for i in range(B_PER):
            for ch in range(C):
                op = OP.bypass if ch == 0 else OP.add
                if ch == 0 and i < CONFIG["sp_c0"]:
                    nc.sync.dma_start(s_tiles[i][:], _plane_view(img_d[i, ch]))
                elif CONFIG["half_dma"]:
                    pv = _plane_view(img_d[i, ch]).rearrange(
                        "p (h f) -> p h f", h=2)
                    for h in range(2):
                        nc.gpsimd.dma_start(s_tiles[i][:, HSL0[h]], pv[:, h],
                                            accum_op=op)
                else:
                    nc.gpsimd.dma_start(
                        s_tiles[i][:], _plane_view(img_d[i, ch]),
                        accum_op=op)

        # ---- helpers ------------------------------------------------------
        def v3(t):
            return t[:].rearrange("p (c w) -> p c w", w=W)

        def row3(eng, out_t, x_t, h, sub=False, edge_eng=None):
            """out = xR - xL (sub) or xL + xR (add) on blocks 2h..2h+1,
            with zero-pad edge fixups (on edge_eng, default gpsimd)."""
            ee = edge_eng or nc.gpsimd
            ov, xv = v3(out_t), v3(x_t)
            c0, c1 = 2 * h, 2 * h + 2
            if sub:
                eng.tensor_sub(ov[:, c0:c1, 1:W - 1],
                               xv[:, c0:c1, 2:W], xv[:, c0:c1, 0:W - 2])
                ee.tensor_copy(ov[:, c0:c1, 0:1], xv[:, c0:c1, 1:2])
                ee.tensor_scalar_mul(ov[:, c0:c1, W - 1:W],
                                     xv[:, c0:c1, W - 2:W - 1], -1.0)
            else:
                eng.tensor_add(ov[:, c0:c1, 1:W - 1],
                               xv[:, c0:c1, 0:W - 2], xv[:, c0:c1, 2:W])
                ee.tensor_copy(ov[:, c0:c1, 0:1], xv[:, c0:c1, 1:2])
                ee.tensor_copy(ov[:, c0:c1, W - 1:W], xv[:, c0:c1, W - 2:W - 1])

        def col_mm(ps_t, inputs, h):
            """H-direction conv taps into psum half-tile ps_t [P, HF].
            inputs: list of (tile, [(dr, coef), ...]).  Out row r = 4p + c
            taps row r + dr: cc = c + dr in 0..3 -> same partition block cc;
            cc == -1 -> lhs(+1) on block 3; cc == 4 -> lhs(-1) on block 0.
            Matmuls are ordered lhs-major (fewer ldweights) while keeping
            start/stop per psum block region correct."""
            c0, c1 = 2 * h, 2 * h + 1
            wides, narrows = [], []  # (lhs, src_ap, regions, out_slice)
            for x_t, taps in inputs:
                xv = x_t[:]
                for dr, coef in taps:
                    cca, ccb = c0 + dr, c1 + dr
                    if CONFIG["wide_mm"] and 0 <= cca and ccb <= RPP - 1:
                        wides.append((lhs(0, coef), xv[:, cca * W:(ccb + 1) * W],
                                      (0, 1), slice(0, 2 * W)))
                        continue
                    for lc, cc in ((0, cca), (1, ccb)):
                        osl = slice(lc * W, (lc + 1) * W)
                        if cc == -1:
                            narrows.append((lhs(+1, coef), xv[:, 3 * W:4 * W],
                                            (lc,), osl))
                        elif cc == RPP:
                            narrows.append((lhs(-1, coef), xv[:, 0:W],
                                            (lc,), osl))
                        else:
                            narrows.append((lhs(0, coef),
                                            xv[:, cc * W:(cc + 1) * W],
                                            (lc,), osl))
            narrows.sort(key=lambda e: (id(e[0]), e[2]))
            todo = wides + narrows
            first = {}; last = {}
            for j, (lt, src, regs, osl) in enumerate(todo):
                for r in regs:
                    first.setdefault(r, j)
                    last[r] = j
            for j, (lt, src, regs, osl) in enumerate(todo):
                st = all(first[r] == j for r in regs)
                sp = all(last[r] == j for r in regs)
                # every region's first writer must carry start; verify
                assert all((first[r] == j) == st for r in regs)
                assert all((last[r] == j) == sp for r in regs)
                nc.tensor.matmul(ps_t[:, osl], lt[:], src, start=st, stop=sp)

        HSL = (slice(0, HF), slice(HF, FW))

        # gauss column taps on r1 (= Brow(s)/G0): fold G0 and the /3 channel
        # mean here.
        m_c = G0 * G0 / 3.0
        c_c = G1 * G0 / 3.0

        # Prebuild all lhsT constants so they are ready before first use.
        for delta, coef in [(+1, m_c), (0, m_c), (0, c_c), (-1, m_c),
                            (+1, 0.5), (0, 0.5), (0, 1.0), (-1, 0.5),
                            (+1, -1.0), (-1, 1.0), (0, -1.0),
                            (+1, -0.5), (-1, 0.5), (0, -0.5),
                            (+1, 1.0), (-1, -0.5), (0, -1.0), (0, 1.0)]:
            lhs(delta, coef)

        # ------------------------------------------------------------------
        # Software-pipelined emission: per-plane work is split into chunks;
        # chunks across planes are emitted sorted by estimated execution
        # time so each in-order sequencer sees its work in roughly the order
        # it becomes runnable (avoids head-of-line blocking).
        # ------------------------------------------------------------------
        state = [dict() for _ in range(B_PER)]

        def ck_a(i):  # gauss row: u_g = sL + sR ; r1 = (G1/G0) s + u_g
            s = s_tiles[i]
            eng = nc.vector if i < 2 else nc.gpsimd
            eeng = nc.vector if i < 2 else None
            u_g = fpool.tile([P, FW], F16, tag="u_g")
            for h in range(2):
                row3(eng, u_g, s, h, edge_eng=eeng)
            r1 = fpool.tile([P, FW], F16, tag="r1")
            for h in range(2):
                nc.vector.scalar_tensor_tensor(
                    r1[:, HSL[h]], s[:, HSL[h]], G1 / G0, u_g[:, HSL[h]],
                    OP.mult, OP.add)
            state[i]["r1"] = r1

        def ck_b(i):  # gauss col (PE) -> b
            r1 = state[i]["r1"]
            b = fpool.tile([P, FW], F16, tag="b")
            for h in range(2):
                ps_b = psum.tile([P, HF], F32, tag="ps")
                if CONFIG["gauss"] == "pd":
                    # psum = m_c * ucol(r1); b = c_c*r1 + psum on DVE
                    col_mm(ps_b, [(r1, [(-1, m_c), (1, m_c)])], h)
                    nc.vector.scalar_tensor_tensor(
                        b[:, HSL[h]], r1[:, HSL[h]], c_c, ps_b[:],
                        OP.mult, OP.add)
                else:
                    col_mm(ps_b, [(r1, [(-1, m_c), (0, c_c), (1, m_c)])], h)
                    nc.scalar.activation(b[:, HSL[h]], ps_b[:], AF.Copy)
            state[i]["b"] = b

        def ck_c(i):  # sobel x: d = bR - bL ; q1 = (col(.5,1,.5) d)^2
            b = state[i]["b"]
            d = fpool.tile([P, FW], F16, tag="d")
            for h in range(2):
                row3(nc.vector, d, b, h, sub=True)
            q1 = fpool.tile([P, FW], F16, tag="q1")
            if CONFIG["sx"] == "pd":
                gx = fpool.tile([P, FW], F16, tag="gx")
                for h in range(2):
                    ps_gx = psum.tile([P, HF], F32, tag="ps")
                    col_mm(ps_gx, [(d, [(-1, 0.5), (1, 0.5)])], h)
                    nc.vector.tensor_add(gx[:, HSL[h]], d[:, HSL[h]], ps_gx[:])
                for h in range(2):
                    nc.scalar.activation(q1[:, HSL[h]], gx[:, HSL[h]], AF.Square)
            else:
                for h in range(2):
                    ps_gx = psum.tile([P, HF], F32, tag="ps")
                    col_mm(ps_gx, [(d, [(-1, 0.5), (0, 1.0), (1, 0.5)])], h)
                    nc.scalar.activation(q1[:, HSL[h]], ps_gx[:], AF.Square)
            state[i]["q1"] = q1

        def ck_d(i):  # sobel y row: u_a = bL + bR
            b = state[i]["b"]
            eng = nc.vector if i < 2 else nc.gpsimd
            eeng = nc.vector if i < 2 else None
            u_a = fpool.tile([P, FW], F16, tag="u_a")
            for h in range(2):
                row3(eng, u_a, b, h, edge_eng=eeng)
            state[i]["u_a"] = u_a

        def ck_e(i):  # sobel y col on (b, u_a); gm = sqrt(q1 + q2)
            b, u_a, q1 = state[i]["b"], state[i]["u_a"], state[i]["q1"]
            q2 = fpool.tile([P, FW], F16, tag="q2")
            if CONFIG["sy"] == "expl":
                # a = 2b + u_a (DVE stt); gy = 0.5*(a[r+1]-a[r-1]): 2 PE taps
                a = fpool.tile([P, FW], F16, tag="a")
                for h in range(2):
                    nc.vector.scalar_tensor_tensor(
                        a[:, HSL[h]], b[:, HSL[h]], 2.0, u_a[:, HSL[h]],
                        OP.mult, OP.add)
                sy_inputs = [(a, [(-1, -0.5), (1, 0.5)])]
            else:
                sy_inputs = [(b, [(-1, -1.0), (1, 1.0)]),
                             (u_a, [(-1, -0.5), (1, 0.5)])]
            for h in range(2):
                ps_gy = psum.tile([P, HF], F32, tag="ps")
                col_mm(ps_gy, sy_inputs, h)
                nc.scalar.activation(q2[:, HSL[h]], ps_gy[:], AF.Square)
            gm2 = fpool.tile([P, FW], F16, tag="gm2")
            for h in range(2):
                nc.vector.tensor_add(gm2[:, HSL[h]], q1[:, HSL[h]], q2[:, HSL[h]])
            gm = fpool.tile([P, FW], F16, tag="gm")
            for h in range(2):
                nc.scalar.activation(gm[:, HSL[h]], gm2[:, HSL[h]], AF.Sqrt)
            state[i]["gm"] = gm

        def ck_f(i):  # ring 1: t1 = S(gm) - gm
            gm = state[i]["gm"]
            u5 = fpool.tile([P, FW], F16, tag="u5")
            for h in range(2):
                row3(nc.vector, u5, gm, h)
            if CONFIG["ring1"][i] == "pair":
                # t1 = Bcol(u5) + ucol(gm): 5 PE taps, Act writeback
                t1 = fpool.tile([P, FW], F16, tag="t1")
                for h in range(2):
                    ps_t = psum.tile([P, HF], F32, tag="ps")
                    col_mm(ps_t, [(u5, [(-1, 1.0), (0, 1.0), (1, 1.0)]),
                                  (gm, [(-1, 1.0), (1, 1.0)])], h)
                    nc.scalar.activation(t1[:, HSL[h]], ps_t[:], AF.Copy)
            elif CONFIG["ring1"][i] == "pair_dve":
                # psum = ucol(u5) + ucol(gm) (4 taps); t1 = u5 + psum on DVE
                t1 = fpool.tile([P, FW], F16, tag="t1")
                for h in range(2):
                    ps_t = psum.tile([P, HF], F32, tag="ps")
                    col_mm(ps_t, [(u5, [(-1, 1.0), (1, 1.0)]),
                                  (gm, [(-1, 1.0), (1, 1.0)])], h)
                    nc.vector.tensor_add(t1[:, HSL[h]], u5[:, HSL[h]], ps_t[:])
            else:  # hyb: t1 = u5 + ucol(br), br = u5 + gm
                br = fpool.tile([P, FW], F16, tag="br")
                for h in range(2):
                    nc.vector.tensor_add(br[:, HSL[h]], u5[:, HSL[h]],
                                         gm[:, HSL[h]])
                ps_up = psbnd.tile([P, W], F32, tag="bnd")  # br[p-1,blk3]->blk0
                nc.tensor.matmul(ps_up[:], lhs(+1, 1.0)[:], br[:, 3 * W:4 * W],
                                 start=True, stop=True)
                ps_dn = psbnd.tile([P, W], F32, tag="bnd")  # br[p+1,blk0]->blk3
                nc.tensor.matmul(ps_dn[:], lhs(-1, 1.0)[:], br[:, 0:W],
                                 start=True, stop=True)
                u6 = fpool.tile([P, FW], F16, tag="u6")
                nc.vector.tensor_add(u6[:, W:3 * W], br[:, 0:2 * W],
                                     br[:, 2 * W:4 * W])
                nc.vector.tensor_add(u6[:, 0:W], br[:, W:2 * W], ps_up[:])
                nc.vector.tensor_add(u6[:, 3 * W:4 * W], br[:, 2 * W:3 * W],
                                     ps_dn[:])
                t1 = fpool.tile([P, FW], F16, tag="t1")
                for h in range(2):
                    nc.vector.tensor_add(t1[:, HSL[h]], u5[:, HSL[h]],
                                         u6[:, HSL[h]])
            state[i]["t1"] = t1

        def ck_g(i):  # ring 2 (PE): out = S(t1) - t1 ; write out
            t1 = state[i]["t1"]
            u7 = fpool.tile([P, FW], F16, tag="u7")
            for h in range(2):
                row3(nc.vector, u7, t1, h)
            o = opool.tile([P, FW], F32, tag="o")
            if CONFIG["ring2"][i] == "pair_dve":
                # psum = ucol(u7) + ucol(t1) (4 taps); o = u7 + psum on DVE
                for h in range(2):
                    ps_o = psum.tile([P, HF], F32, tag="ps")
                    col_mm(ps_o, [(u7, [(-1, 1.0), (1, 1.0)]),
                                  (t1, [(-1, 1.0), (1, 1.0)])], h)
                    nc.vector.tensor_add(o[:, HSL[h]], u7[:, HSL[h]], ps_o[:])
            else:
                if CONFIG["ring2"][i] == "pair":
                    inputs = [(u7, [(-1, 1.0), (0, 1.0), (1, 1.0)]),
                              (t1, [(-1, 1.0), (1, 1.0)])]
                else:  # expl: br2 = u7 + t1
                    br2 = fpool.tile([P, FW], F16, tag="br2")
                    for h in range(2):
                        nc.vector.tensor_add(br2[:, HSL[h]], u7[:, HSL[h]],
                                             t1[:, HSL[h]])
                    inputs = [(br2, [(-1, 1.0), (0, 1.0), (1, 1.0)]),
                              (t1, [(0, -1.0)])]
                for h in range(2):
                    ps_o = psum.tile([P, HF], F32, tag="ps")
                    col_mm(ps_o, inputs, h)
                    nc.scalar.activation(o[:, HSL[h]], ps_o[:], AF.Copy)
            for h in range(2):
                nc.sync.dma_start(
                    _plane_view(out_d[i]).rearrange("p (h f) -> p h f", h=2)[:, h],
                    o[:, HSL[h]])

        chunks = [ck_a, ck_b, ck_c, ck_d, ck_e, ck_f, ck_g]
        # est start (us): s-ready stagger (planes arrive in pairs) + chain
        s_ready = CONFIG.get("s_ready", [17.5, 20.0, 32.0, 35.0])
        dur = CONFIG.get("dur", [2.0, 3.0, 3.5, 1.0, 4.0, 5.0, 5.5])
        sched = []
        for i in range(B_PER):
            t = s_ready[i]
            for k, ck in enumerate(chunks):
                sched.append((t, i, k))
                t += dur[k]
        sched.sort()
        for t, i, k in sched:
            chunks[k](i)

    nc.compile()
    return nc


_NC = None


def _get_nc():
    global _NC
    if _NC is None:
        _NC = _build_nc()
    return _NC


def kernel(**inputs):
    img = np.ascontiguousarray(np.asarray(inputs["img"], dtype=np.float32))
    nc = _get_nc()
    in_maps = [{"img": img[B_PER * c:B_PER * (c + 1)]} for c in range(N_CORES)]
    res = run_bass_kernel_spmd(nc, in_maps, list(range(N_CORES)))
    out = np.concatenate([res.results[c]["out"] for c in range(N_CORES)], axis=0)
    return out[:, None, :, :]
